# revision 6
# baseline (speedup 1.0000x reference)
"""Allegro-style equivariant GNN edge-network on 8 TRN2 NeuronCores — v5.

N=1024 chunks (DVE/ACT fixed overheads amortize), 2-way software pipeline.
Per-chunk PSUM footprint merged into two [128,1024] tiles (TA: embed/MLP/n0
groups, TB: env/G1 groups) so two chunks fit in the 8 PSUM banks. The layer-0
t000 path is folded through scal0 (Wt000s = W_env @ (w0*Wlin0)), which
removes the env128 SBUF evacuation: the xall/xb products read env directly
from PSUM (valid because benv==0 per the spec; checked at build time).
"""

import sys

sys.path.insert(0, "/opt/trn_rl_repo")

import numpy as np
import ml_dtypes

BF = ml_dtypes.bfloat16

import concourse.bass as bass
import concourse.mybir as mybir
from concourse import bacc
from concourse.tile import TileContext
from concourse.bass_utils import run_bass_kernel_spmd

E = 131072
NCORES = 8
EC = E // NCORES
C = 16
S = 64
NB = 8
TE = 16
NL = 2
RMAX = 5.0

N = 1024
NCHUNK = EC // N   # 16
NS = 2             # chunks in flight
ROUNDS = NCHUNK // NS
H = N // 512

F32 = mybir.dt.float32
BF16 = mybir.dt.bfloat16


def _Qnp():
    Q = np.zeros((5, 3, 3))
    s = 1.0 / np.sqrt(2.0)
    Q[0, 0, 1] = Q[0, 1, 0] = s
    Q[1, 1, 2] = Q[1, 2, 1] = s
    Q[2] = np.diag([-1.0, -1.0, 2.0]) / np.sqrt(6.0)
    Q[3, 0, 2] = Q[3, 2, 0] = s
    Q[4] = np.diag([1.0, -1.0, 0.0]) * s
    return Q


_Q = _Qnp()
_An = np.einsum('mij,pjk,qki->mpq', _Q, _Q, _Q)
_A = 0.5 * (_An + _An.transpose(0, 2, 1))


def _fold_weights(inp):
    f = lambda a: np.ascontiguousarray(a, dtype=np.float32)
    W = {}
    B = {}
    s0 = 1.0 / np.sqrt(3.0 * C)
    s1 = 1.0 / np.sqrt(4.0 * C)
    s2 = 1.0 / np.sqrt(4.0 * C)

    W["We1"] = f(inp["W_e1"])
    B["be1"] = f(inp["b_e1"].reshape(S, 1))
    W["We2"] = f(inp["W_e2"])
    B["be2"] = f(inp["b_e2"].reshape(S, 1))

    env_srcs = [
        (np.asarray(inp["W_env_e"], np.float64), np.asarray(inp["b_env_e"], np.float64)),
        (np.asarray(inp["Wenv"][0], np.float64), np.asarray(inp["benv"][0], np.float64)),
    ]
    for t, (We, be) in enumerate(env_srcs):
        w = np.zeros((S, 128), np.float64)
        b = np.zeros((128, 1), np.float64)
        for m in range(5):
            w[:, m * C:(m + 1) * C] = We
            b[m * C:(m + 1) * C, 0] = be
        for i in range(3):
            w[:, 80 + i * C:80 + (i + 1) * C] = We
            b[80 + i * C:80 + (i + 1) * C, 0] = be
        W[f"Wenv128_{t}"] = f(w)
        B[f"benv128_{t}"] = f(b)

    for l in range(NL):
        W[f"Wm1f_{l}"] = f(inp["Wm1"][l])
        B[f"bm1_{l}"] = f(inp["bm1"][l].reshape(S, 1))
        W[f"Wm2_{l}"] = f(inp["Wm2"][l])
        B[f"bm2_{l}"] = f(inp["bm2"][l].reshape(S, 1))

    w = np.asarray(inp["w_tp"][0], np.float64)
    W0 = np.asarray(inp["Wlin0"][0], np.float64)
    W1 = np.asarray(inp["Wlin1"][0], np.float64)
    W2 = np.asarray(inp["Wlin2"][0], np.float64)

    wx = np.zeros((128, 128), np.float64)
    for m in range(5):
        for c in range(C):
            wx[m * C + c, m * C:(m + 1) * C] = (
                w[2][c] * W2[c] + w[7][c] * W2[32 + c]) * s2
    for i in range(3):
        for c in range(C):
            wx[80 + i * C + c, 80 + i * C:80 + (i + 1) * C] = (
                w[1][c] * W1[c] + w[3][c] * W1[16 + c]) * s1
    W["WX"] = f(wx)

    for j in range(3):
        wm = np.zeros((128, 128), np.float64)
        for m in range(5):
            for i in range(3):
                for c in range(C):
                    wm[m * C + c, 80 + i * C:80 + (i + 1) * C] += _Q[m, i, j] * (
                        w[6][c] * W1[32 + c] + w[8][c] * W1[48 + c]) * s1
        for i in range(3):
            for m in range(5):
                for c in range(C):
                    wm[80 + i * C + c, m * C:(m + 1) * C] += (
                        _Q[m, i, j] * w[5][c] * W2[16 + c] * s2)
        W[f"WM_{j}"] = f(wm)

    for q in range(5):
        wq = np.zeros((80, 80), np.float64)
        for p in range(5):
            for m in range(5):
                for c in range(C):
                    wq[p * C + c, m * C:(m + 1) * C] += (
                        _A[m, p, q] * w[10][c] * W2[48 + c] * s2)
        W[f"WP4_{q}"] = f(wq)

    # t000 folded through scal0: env16 = We.T@scal0 + be
    We, be = env_srcs[0]
    W["Wt000s_0"] = f(We @ (w[0][:, None] * W0[0:16]) * s0)        # (64,16)
    B["bn0_0"] = f(((w[0][:, None] * W0[0:16]) * s0).T @ be.reshape(16, 1))
    wp = np.zeros((128, 16), np.float64)
    for m in range(5):
        for c in range(C):
            wp[m * C + c] = w[9][c] * W0[32 + c] * s0
    for i in range(3):
        for c in range(C):
            wp[80 + i * C + c] = w[4][c] * W0[16 + c] * s0
    W["WPdG2"] = f(wp)

    w = np.asarray(inp["w_tp"][1], np.float64)
    W0 = np.asarray(inp["Wlin0"][1], np.float64)
    wp = np.zeros((128, 16), np.float64)
    for m in range(5):
        for c in range(C):
            wp[m * C + c] = w[9][c] * W0[32 + c] * s0
    for i in range(3):
        for c in range(C):
            wp[80 + i * C + c] = w[4][c] * W0[16 + c] * s0
    W["WPdb1"] = f(wp)
    W["Wt000_1"] = f(w[0][:, None] * W0[0:16] * s0)
    B["bz16"] = np.zeros((16, 1), np.float32)

    return W, B


def _pack_weights(W, B):
    offs = {}
    col = 0
    items = []
    for nm, a in W.items():
        row = 64 if nm == "Wt000_1" else 0
        k, m = a.shape
        offs[nm] = (row, k, m, col)
        items.append((nm, a, row))
        col += m
    for nm, a in B.items():
        k, m = a.shape
        offs[nm] = (0, k, m, col)
        items.append((nm, a, 0))
        col += m
    arr = np.zeros((128, col), BF)
    for nm, a, row in items:
        _, k, m, o = offs[nm]
        arr[row:row + k, o:o + m] = a.astype(BF)
    return arr, offs


def _build_nc(woffs, wcols):
    nc = bacc.Bacc()
    h_p = nc.declare_dram_parameter("h", [24, EC], BF16, isOutput=False)
    dg_p = nc.declare_dram_parameter("dgeom", [1, 3 * EC], BF16, isOutput=False)
    yg_p = nc.declare_dram_parameter("ygeom", [1, 5 * EC], BF16, isOutput=False)
    wpack_p = nc.declare_dram_parameter("wpack", [128, wcols], BF16, isOutput=False)
    out_p = nc.declare_dram_parameter("out", [NL, S, EC], BF16, isOutput=True)

    h_ap = h_p[:]
    dg_ap = dg_p[:]
    yg_ap = yg_p[:]
    out_ap = out_p[:]

    def dsrc(offset, pattern):
        return bass.AP(tensor=dg_ap.tensor, offset=offset, ap=pattern)

    def ysrc(offset, pattern):
        return bass.AP(tensor=yg_ap.tensor, offset=offset, ap=pattern)

    ACT = mybir.ActivationFunctionType

    with TileContext(nc) as tc:
        with (
            tc.tile_pool(name="const", bufs=1) as constp,
            tc.tile_pool(name="geo", bufs=2) as geo,
            tc.tile_pool(name="work", bufs=1) as work,
            tc.tile_pool(name="psum", bufs=1, space="PSUM") as psump,
        ):
            wpack = constp.tile([128, wcols], BF16, name="wpack", tag="wpack")
            nc.sync.dma_start(out=wpack, in_=wpack_p[:])

            def wt(nm):
                row, k, m, o = woffs[nm]
                return wpack[row:row + k, o:o + m]

            def ps(tag):
                return psump.tile([128, N], F32, name=tag, tag=tag)

            warm0 = ps("TA_0")
            nc.tensor.matmul(warm0[:1, :1], wpack[:1, :1], wpack[:1, :1],
                             start=True, stop=True)
            for wi in range(12):
                nc.tensor.matmul(warm0[:, :512], wpack[:128, :128],
                                 wpack[:128, 512:1024], start=True, stop=True)

            def mmh(out, w_, r_, start, stop, tp=None):
                for h in range(H):
                    hs = slice(h * 512, (h + 1) * 512)
                    nc.tensor.matmul(out[:, hs], w_, r_[:, hs], start=start,
                                     stop=stop, tile_position=tp,
                                     skip_group_check=True)

            def wtile(s, rows, nm, cols=N):
                return work.tile([rows, cols], BF16, name=f"{nm}{s}",
                                 tag=f"{nm}{s}")

            st = [dict() for _ in range(NS)]

            def g_dma(ch, s):
                o = ch * N
                c = st[s]
                c["hT"] = geo.tile([24, N], BF16, name=f"hT{s}", tag=f"hT{s}")
                nc.sync.dma_start(out=c["hT"], in_=h_ap[:, o:o + N])
                c["g128"] = geo.tile([128, N], BF16, name=f"g{s}", tag=f"g{s}")
                nc.gpsimd.dma_start(
                    out=c["g128"][0:80],
                    in_=ysrc(5 * o, [[N, 5], [0, 16], [1, N]]))
                nc.gpsimd.dma_start(
                    out=c["g128"][80:128],
                    in_=dsrc(3 * o, [[N, 3], [0, 16], [1, N]]))
                c["dj"] = geo.tile([128, 3 * N], BF16, name=f"dj{s}", tag=f"dj{s}")
                nc.sync.dma_start(
                    out=c["dj"], in_=dsrc(3 * o, [[0, 128], [1, 3 * N]]))
                c["ym"] = geo.tile([80, 5 * N], BF16, name=f"ym{s}", tag=f"ym{s}")
                nc.sync.dma_start(
                    out=c["ym"], in_=ysrc(5 * o, [[0, 80], [1, 5 * N]]))

            def g_pe1(ch, s):
                c = st[s]
                c["TA"] = ps(f"TA_{s}")
                mmh(c["TA"][0:64], wt("We1"), c["hT"], True, True)

            def warm(c):
                # parasitic matmul into the stream's currently-idle TB tile:
                # keeps the PE HAM window busy through serial ACT phases
                nc.tensor.matmul(c["TB"][:, :512], wpack[:128, :128],
                                 wpack[:128, 512:1024], start=True, stop=True)

            def g_sb1(ch, s):
                c = st[s]
                c["sb1"] = wtile(s, 64, "sb1")
                nc.scalar.activation(c["sb1"], c["TA"][0:64], ACT.Silu,
                                     bias=wt("be1"))
                if "TB" in c:
                    warm(c)

            def g_pe2(ch, s):
                c = st[s]
                mmh(c["TA"][0:64], wt("We2"), c["sb1"], True, True)

            def g_scal0(ch, s):
                c = st[s]
                c["comb0"] = wtile(s, 80, "comb0")
                nc.scalar.activation(c["comb0"][0:64], c["TA"][0:64], ACT.Silu,
                                     bias=wt("be2"))
                if "TB" in c:
                    warm(c)

            def g_penv0(ch, s):
                c = st[s]
                c["TB"] = ps(f"TB_{s}")
                mmh(c["TB"], wt("Wenv128_0"), c["comb0"][0:64], True, True)

            def g_xall(ch, s):
                c = st[s]
                c["xall"] = wtile(s, 128, "xall")
                nc.vector.tensor_mul(c["xall"], c["TB"], c["g128"])

            def g_prod1(ch, s):
                c = st[s]
                c["Pd"] = wtile(s, 128, "Pd")
                nc.vector.tensor_mul(c["Pd"], c["xall"], c["g128"])
                # all three M_j in one DVE op: xall repeated 3x via stride-0 AP
                xa = c["xall"]
                pitch = xa.ap[0][0]
                rep3 = bass.AP(tensor=xa.tensor, offset=xa.offset,
                               ap=[[pitch, 128], [0, 3], [1, N]])
                c["Mall"] = wtile(s, 128, "Mall", cols=3 * N)
                nc.vector.tensor_mul(c["Mall"], rep3, c["dj"])

            def g_prod2(ch, s):
                c = st[s]
                xa = c["xall"]
                pitch = xa.ap[0][0]
                rep5 = bass.AP(tensor=xa.tensor, offset=xa.offset,
                               ap=[[pitch, 80], [0, 5], [1, N]])
                c["P4all"] = wtile(s, 80, "P4all", cols=5 * N)
                nc.vector.tensor_mul(c["P4all"], rep5, c["ym"])

            def g_G1a(ch, s):
                c = st[s]
                mmh(c["TB"][:128], wt("WX"), c["xall"], True, False)
                for j in range(3):
                    mmh(c["TB"][:128], wt(f"WM_{j}"),
                        c["Mall"][:, j * N:(j + 1) * N], False, False)

            def g_G2(ch, s):
                c = st[s]
                mmh(c["TA"][64:80], wt("Wt000s_0"), c["comb0"][0:64],
                    True, False, tp=(0, 64))
                mmh(c["TA"][64:80], wt("WPdG2"), c["Pd"],
                    False, True, tp=(0, 64))

            def g_G1b(ch, s):
                c = st[s]
                for q in range(5):
                    mmh(c["TB"][:80], wt(f"WP4_{q}"),
                        c["P4all"][:, q * N:(q + 1) * N], False, q == 4)

            def g_evacG(ch, s):
                c = st[s]
                c["n128"] = wtile(s, 128, "n128")
                nc.vector.tensor_copy(c["n128"], c["TB"])

            def g_n0(ch, s):
                c = st[s]
                nc.scalar.activation(c["comb0"][64:80], c["TA"][64:80],
                                     ACT.Identity, bias=wt("bn0_0"))

            def g_pm1(ch, s):
                c = st[s]
                mmh(c["TA"][0:64], wt("Wm1f_0"), c["comb0"], True, True)

            def g_mh0(ch, s):
                c = st[s]
                c["mh0"] = wtile(s, 64, "mh0")
                nc.scalar.activation(c["mh0"], c["TA"][0:64], ACT.Silu,
                                     bias=wt("bm1_0"))
                warm(c)

            def g_pm2(ch, s):
                c = st[s]
                mmh(c["TA"][0:64], wt("Wm2_0"), c["mh0"], True, True)

            def g_scal1(ch, s):
                o = ch * N
                c = st[s]
                warm(c)
                c["comb1"] = wtile(s, 80, "comb1")
                nc.scalar.activation(c["comb1"][0:64], c["TA"][0:64],
                                     ACT.Identity, bias=wt("bm2_0"))
                nc.sync.dma_start(out=out_ap[0, :, o:o + N], in_=c["comb1"][0:64])

            def g_penv1(ch, s):
                c = st[s]
                mmh(c["TB"], wt("Wenv128_1"), c["comb1"][0:64], True, True)

            def g_env1(ch, s):
                c = st[s]
                c["env128b"] = wtile(s, 128, "env128b")
                nc.scalar.activation(c["env128b"], c["TB"], ACT.Identity,
                                     bias=wt("benv128_1"))

            def g_prodb(ch, s):
                c = st[s]
                c["xb"] = wtile(s, 128, "xb")
                nc.vector.tensor_mul(c["xb"], c["n128"], c["env128b"])
                c["Pdb"] = wtile(s, 128, "Pdb")
                nc.vector.tensor_mul(c["Pdb"], c["xb"], c["g128"])
                c["x0t"] = wtile(s, 80, "x0t")
                nc.vector.tensor_mul(c["x0t"][64:80], c["comb0"][64:80],
                                     c["env128b"][64:80])

            def g_G2b(ch, s):
                c = st[s]
                mmh(c["TA"][64:80], wt("WPdb1"), c["Pdb"],
                    True, False, tp=(0, 64))
                mmh(c["TA"][64:80], wt("Wt000_1"), c["x0t"][64:80],
                    False, True, tp=(64, 64))

            def g_n0b(ch, s):
                c = st[s]
                nc.scalar.activation(c["comb1"][64:80], c["TA"][64:80],
                                     ACT.Identity, bias=wt("bz16"))
                warm(c)

            def g_pm1b(ch, s):
                c = st[s]
                mmh(c["TA"][0:64], wt("Wm1f_1"), c["comb1"], True, True)

            def g_mh1(ch, s):
                c = st[s]
                c["mh1"] = wtile(s, 64, "mh1")
                nc.scalar.activation(c["mh1"], c["TA"][0:64], ACT.Silu,
                                     bias=wt("bm1_1"))
                warm(c)

            def g_pm2b(ch, s):
                c = st[s]
                mmh(c["TA"][0:64], wt("Wm2_1"), c["mh1"], True, True)

            def g_scal2(ch, s):
                o = ch * N
                c = st[s]
                warm(c)
                c["scal2t"] = wtile(s, 64, "scal2t")
                nc.scalar.activation(c["scal2t"], c["TA"][0:64], ACT.Identity,
                                     bias=wt("bm2_1"))
                nc.sync.dma_start(out=out_ap[1, :, o:o + N], in_=c["scal2t"])

            GROUPS = [g_dma, g_pe1, g_sb1, g_pe2, g_scal0, g_penv0, g_xall,
                      g_prod1, g_prod2, g_G1a, g_G2, g_G1b, g_evacG,
                      g_n0, g_pm1, g_mh0, g_pm2, g_scal1, g_penv1, g_env1,
                      g_prodb, g_G2b, g_n0b, g_pm1b, g_mh1, g_pm2b, g_scal2]

            # Stream B lags stream A by half a chunk's groups so B's G1
            # matmul bursts fill A's serial MLP phases (and vice versa).
            tasksA = [(g, r * NS + 0) for r in range(ROUNDS) for g in GROUPS]
            tasksB = [(g, r * NS + 1) for r in range(ROUNDS) for g in GROUPS]
            OFF = len(GROUPS) // 2
            for i in range(len(tasksA) + OFF):
                if i < len(tasksA):
                    g, ch = tasksA[i]
                    g(ch, 0)
                j = i - OFF
                if 0 <= j < len(tasksB):
                    g, ch = tasksB[j]
                    g(ch, 1)
    nc.finalize()
    return nc


_NC_CACHE = None


def _host_prep(inputs):
    bond_dist = np.asarray(inputs["bond_dist"], np.float32)
    bond_diff = np.asarray(inputs["bond_diff"], np.float32)
    emb = np.asarray(inputs["emb_table"], np.float32)
    Z = np.asarray(inputs["Z"]).astype(np.int64)
    ei = np.asarray(inputs["edge_index"]).astype(np.int64)

    u = bond_dist / RMAX
    n = np.arange(1, NB + 1, dtype=np.float32)
    radial = (np.sqrt(np.float32(2.0 / RMAX)) *
              np.sin(np.float32(np.pi) * n * u[:, None].astype(np.float32)) /
              bond_dist[:, None])
    cutoff = np.where(u < 1.0, 1.0 - 28.0 * u**6 + 48.0 * u**7 - 21.0 * u**8, 0.0)
    radial = (radial * cutoff[:, None].astype(np.float32)).astype(np.float32)

    d = (bond_diff / (bond_dist[:, None] + np.float32(1e-8))).astype(np.float32)
    y2 = (np.sqrt(np.float32(1.5)) *
          np.einsum('mij,ei,ej->em', _Q.astype(np.float32), d, d)).astype(np.float32)

    te = (emb[Z[ei[:, 0]]] * emb[Z[ei[:, 1]]]).astype(np.float32)

    h = np.ascontiguousarray(np.concatenate([radial, te], axis=1).T.astype(BF))
    dT = d.T.reshape(3, NCORES, NCHUNK, N)
    yT = y2.T.reshape(5, NCORES, NCHUNK, N)
    dgeom = np.ascontiguousarray(
        dT.transpose(1, 2, 0, 3).reshape(NCORES, 3 * EC).astype(BF))
    ygeom = np.ascontiguousarray(
        yT.transpose(1, 2, 0, 3).reshape(NCORES, 5 * EC).astype(BF))
    W, Bd = _fold_weights(inputs)
    return h, dgeom, ygeom, W, Bd


def make_in_maps(inputs):
    global _NC_CACHE
    h, dgeom, ygeom, W, Bd = _host_prep(inputs)
    # xall/xb read env from PSUM pre-bias; valid only with zero env biases
    # (guaranteed by the spec: benv/b_env_e fill is zeros)
    assert np.all(Bd["benv128_0"] == 0), \
        "nonzero layer-0 env bias unsupported by fused PSUM-read path"
    wpack, woffs = _pack_weights(W, Bd)
    if _NC_CACHE is None:
        _NC_CACHE = _build_nc(woffs, wpack.shape[1])
    in_maps = []
    for i in range(NCORES):
        sl = slice(i * EC, (i + 1) * EC)
        m = {"h": np.ascontiguousarray(h[:, sl]),
             "dgeom": dgeom[i].reshape(1, 3 * EC),
             "ygeom": ygeom[i].reshape(1, 5 * EC),
             "wpack": wpack}
        in_maps.append(m)
    return in_maps


def kernel(**inputs):
    in_maps = make_in_maps(inputs)
    res = run_bass_kernel_spmd(_NC_CACHE, in_maps, list(range(NCORES))).results
    out = np.concatenate(
        [np.asarray(res[i]["out"]).astype(np.float32).transpose(2, 0, 1)
         for i in range(NCORES)], axis=0)
    return np.ascontiguousarray(out)
